# revision 39
# baseline (speedup 1.0000x reference)
"""AlphaWeightedHead Trainium2 kernel: per-sample sigmoid-gated QKV + MHA + proj.

Sharding: data-parallel over batch, 2 samples per core x 8 cores.
All device tensors use a feature-major ("transposed") layout so every matmul
reads its operands in natural orientation (no on-device transposes):

  x^T [c, t]  -> QKV^T [d, t] (Q/K) and V [t, hv]   (contraction over c)
  S^T [tk,tq] = K^T.T @ Q^T per head (contraction over hd, row-packed 2 heads)
  P^T = exp(S^T * scale)  (no max-subtract: |scores| < ~1, data-bounded)
  O^T_aug [65, tq] = [V | ones].T @ P^T  (row 64 = softmax denominator;
      the ones column is folded into the padded V weights host-side)
  Y^T [c_out, t] = pw^T.T @ (O^T / denom)

All matmuls run in bf16 (PSUM accumulates fp32). Host precomputes
sigmoid(alpha[label]), pre-scales V weights/biases, and pads V with the
ones column per head (wv zero-col + bias 1.0 -> V-tile column of ones).

v2 pipeline: the scalar-engine exp stream (~25M elements/core) is the
critical path, so everything else is scheduled around it. The V projection
and the non-urgent QK tiles are emitted as deadline-tagged "fillers"
interleaved between S-matmul groups, so the first exp fires early instead
of after the whole V+QK prologue. Softmax normalization uses
reciprocal_approx_fast on the PSUM denominator row + a DMA
partition-broadcast (the exact DVE reciprocal costs ~8 cyc/elem on a single
partition and serialized the PV chain), and the output projection is
emitted per token-tile as soon as the last head pair finishes that tile.
"""

import sys

import numpy as np
import ml_dtypes
from contextlib import ExitStack

try:
    import concourse.bass as _probe  # noqa: F401
except ModuleNotFoundError:
    sys.path.insert(0, "/opt/trn_rl_repo")

import concourse.bass as bass
import concourse.bacc as bacc
import concourse.tile as tile
from concourse import mybir
from concourse.bass_utils import run_bass_kernel_spmd

B, NT, C, H, CLS = 16, 1024, 768, 12, 1000
HD = 64
NCORES = 8
SPC = B // NCORES          # samples per core = 2
T = SPC * NT               # tokens per core = 2048
NPAIR = H // 2             # 6 head pairs
CH = C // 128              # 6 contraction chunks
CP = H * (HD + 1)          # padded V width = 780 (65 per head)
SCALE = HD ** -0.5
LAG = 3                    # PV trails exp by LAG (s,tq) iterations

F32 = mybir.dt.float32
BF16 = mybir.dt.bfloat16
ADD = mybir.AluOpType.add
MULT = mybir.AluOpType.mult
BYPASS = mybir.AluOpType.bypass
EXP = mybir.ActivationFunctionType.Exp


def build(debug=False, phases="all", reps=1, legacy_pv=False, norm="chain"):
    nc = bacc.Bacc("TRN2")
    xt = nc.declare_dram_parameter("xt", [C, T], BF16, isOutput=False)
    wqk = nc.declare_dram_parameter("wqk", [C, 2 * C], BF16, isOutput=False)
    wv = nc.declare_dram_parameter("wv", [SPC, C, CP], BF16, isOutput=False)
    sigbq = nc.declare_dram_parameter("sigbq", [128, 2 * SPC * 12], F32,
                                      isOutput=False)
    bvs = nc.declare_dram_parameter("bvs", [SPC, CP], BF16, isOutput=False)
    pw = nc.declare_dram_parameter("pw", [C, C], BF16, isOutput=False)
    pb = nc.declare_dram_parameter("pb", [128, CH], F32, isOutput=False)
    out = nc.declare_dram_parameter("out", [C, T], F32, isOutput=True)

    with tile.TileContext(nc) as tc, ExitStack() as ctx:
        cpool = ctx.enter_context(tc.tile_pool(name="const", bufs=1))
        wvp = ctx.enter_context(tc.tile_pool(name="wvp", bufs=2))
        wqkp = ctx.enter_context(tc.tile_pool(name="wqkp", bufs=2))
        qkp = ctx.enter_context(tc.tile_pool(name="qkp", bufs=2))
        ptp = ctx.enter_context(tc.tile_pool(name="ptp", bufs=2 * (LAG + 1)))
        stgp = ctx.enter_context(tc.tile_pool(name="stgp", bufs=2))
        rqp = ctx.enter_context(tc.tile_pool(name="rqp", bufs=2))
        yp = ctx.enter_context(tc.tile_pool(name="yp", bufs=2))
        dramp = ctx.enter_context(
            tc.tile_pool(name="dramp", bufs=2, space=bass.MemorySpace.DRAM))
        # PSUM: 8 banks total = stps 2x2 + mmps 2 + pvps 2
        mmps = ctx.enter_context(
            tc.tile_pool(name="mmps", bufs=2, space=bass.MemorySpace.PSUM))
        stps = ctx.enter_context(
            tc.tile_pool(name="stps", bufs=2, space=bass.MemorySpace.PSUM))
        pvps = ctx.enter_context(
            tc.tile_pool(name="pvps", bufs=2, space=bass.MemorySpace.PSUM))

        def emit_rep():
            # ---- resident tensors (sample-0 columns of x first, so the
            # first S-matmuls can start as early as possible)
            xt_sb = cpool.tile([128, CH, T], BF16)
            wqk0_dma = []

            def emit_xt(half):
                for c in range(CH):
                    nc.sync.dma_start(
                        xt_sb[:, c, half * NT:(half + 1) * NT],
                        xt[c * 128:(c + 1) * 128, half * NT:(half + 1) * NT])

            emit_xt(0)
            emit_xt(1)
            sigbq_sb = cpool.tile([128, 2 * SPC * 12], F32)
            nc.sync.dma_start(sigbq_sb[:], sigbq[:])
            # big non-urgent loads go out on the ACT engine's DMA queue
            # (it only carries exp work later) so they don't delay wqk/xt on
            # sync's queue; measured faster than using the Pool queue, which
            # carries the normalize-chain DMAs
            pw_sb = cpool.tile([128, CH, C], BF16)
            nc.scalar.dma_start(pw_sb[:], pw.rearrange("(c p) n -> p c n", p=128))
            pb_sb = cpool.tile([128, CH], F32)
            nc.scalar.dma_start(pb_sb[:], pb[:])
            bvs_sb = cpool.tile([128, SPC, CP], BF16)
            for s in range(SPC):
                nc.scalar.dma_start(
                    bvs_sb[:, s:s + 1, :],
                    bvs[s:s + 1, :].partition_broadcast(128))
            wv_sb = []
            for s in range(SPC):
                w = wvp.tile([128, CH, CP], BF16, tag="wv", name=f"wv{s}")
                nc.scalar.dma_start(
                    w[:], wv[s].rearrange("(c p) n -> p c n", p=128))
                wv_sb.append(w)
            ob = cpool.tile([128, CH, T], BF16)
            vbs = [cpool.tile([128, 8, CP], BF16, tag=f"vb{s}", name=f"vb{s}")
                   for s in range(SPC)]
            ones_t = cpool.tile([128, 64], BF16)
            nc.vector.memset(ones_t[64:65, :], 1.0)
            rbb_c = None
            if norm == "const":
                rbb_c = cpool.tile([128, 512], F32, tag="rbbc", name="rbbc")
                nc.vector.memset(rbb_c[:], 0.001)

            # ---- emission helpers
            def v_block(s, tt):
                # V_pad[t, 780] = x_s @ wv_pad + bvs_pad, one 128-token tile
                for hvt in range(2):
                    h0 = hvt * 512
                    hvn = 512 if hvt == 0 else CP - 512
                    ps = mmps.tile([128, 512], F32, tag="mm")
                    for c in range(CH):
                        nc.tensor.matmul(
                            ps[:, :hvn],
                            xt_sb[:, c,
                                  s * NT + tt * 128: s * NT + (tt + 1) * 128],
                            wv_sb[s][:, c, h0: h0 + hvn],
                            start=(c == 0), stop=(c == CH - 1),
                        )
                    nc.vector.tensor_add(
                        vbs[s][:, tt, h0:h0 + hvn], ps[:, :hvn],
                        bvs_sb[:, s, h0:h0 + hvn])

            def qk_group(p, wqk_t, qk_t, qk, n):
                ps = mmps.tile([128, 512], F32, tag="mm")
                for c in range(CH):
                    nc.tensor.matmul(
                        ps[:],
                        wqk_t[:, c, qk * 128:(qk + 1) * 128],
                        xt_sb[:, c, n * 512:(n + 1) * 512],
                        start=(c == 0), stop=(c == CH - 1),
                    )
                s = n // 2
                j = s * 12 + qk * 6 + p
                nc.vector.tensor_scalar(
                    qk_t[:, qk, n * 512:(n + 1) * 512], ps[:],
                    sigbq_sb[:, j:j + 1], sigbq_sb[:, 24 + j:24 + j + 1],
                    MULT, ADD)

            # deadline-tagged filler queue: (window_deadline, fn)
            fillers = []

            def drain(k=None, upto=None):
                while fillers:
                    if k is not None and k <= 0:
                        break
                    if upto is not None and fillers[0][0] > upto:
                        break
                    fillers.pop(0)[1]()
                    if k is not None:
                        k -= 1

            def wnum(p, s, tq):
                return 4 * p + 2 * s + tq + 1

            def emit_pv_legacy(item):
                p, s, tq, pt = item
                for hh in range(2):
                    h = 2 * p + hh
                    pv = pvps.tile([128, 512], F32, tag="pv", name="pv")
                    for chk in range(8):
                        nc.tensor.matmul(
                            pv[0:65, :],
                            vbs[s][:, chk, h * 65: h * 65 + 65],
                            pt[hh][:, chk, :],
                            start=(chk == 0), stop=(chk == 7),
                        )
                    stg = stgp.tile([128, 512], BF16, tag="stg", name="stg")
                    nc.vector.tensor_copy(stg[0:65, :], pv[0:65, :])
                    with nc.allow_low_precision(reason="softmax denom bf16"):
                        nc.vector.reciprocal(stg[64:65, :], stg[64:65, :])
                    rb = pvps.tile([128, 512], F32, tag="pv", name="rb")
                    nc.tensor.matmul(
                        rb[0:64, :], ones_t[64:65, :], stg[64:65, :],
                        start=True, stop=True, tile_position=(64, 0))
                    win = slice(s * NT + tq * 512, s * NT + (tq + 1) * 512)
                    if hh == 0:
                        nc.vector.scalar_tensor_tensor(
                            ob[0:64, p, win], stg[0:64, :], 0.0, rb[0:64, :],
                            BYPASS, MULT)
                    else:
                        stn = stgp.tile([64, 512], BF16, tag="stn", name="stn")
                        nc.vector.scalar_tensor_tensor(
                            stn[:], stg[0:64, :], 0.0, rb[0:64, :],
                            BYPASS, MULT)
                        nc.gpsimd.dma_start(ob[64:128, p, win], stn[:])

            def emit_pv_v2_hh(item, hh):
                p, s, tq, pt = item
                h = 2 * p + hh
                pv = pvps.tile([128, 512], F32, tag="pv", name="pv")
                for chk in range(8):
                    nc.tensor.matmul(
                        pv[0:65, :],
                        vbs[s][:, chk, h * 65: h * 65 + 65],
                        pt[hh][:, chk, :],
                        start=(chk == 0), stop=(chk == 7),
                    )
                if norm == "none":
                    return
                if norm == "const":
                    rbb = rbb_c
                else:
                    # reciprocal_approx_fast (custom-DVE) only works at base
                    # partition 0, so shift the denominator row there with a
                    # plain copy (plain DVE copies can partition-shift)
                    rr = rqp.tile([128, 512], F32, tag="rr", name="rr")
                    nc.vector.tensor_copy(rr[0:1, :], pv[64:65, :])
                    rq2 = rqp.tile([128, 512], F32, tag="rq2", name="rq2")
                    nc.vector.reciprocal_approx_fast(rq2[0:1, :], rr[0:1, :])
                    # SBUF APs can't partition-broadcast (zero step), so
                    # bounce the reciprocal row through DRAM
                    rr_d = dramp.tile([1, 512], F32, tag="rrd", name="rrd")
                    nc.gpsimd.dma_start(rr_d[:], rq2[0:1, :])
                    rbb = rqp.tile([128, 512], F32, tag="rbb", name="rbb")
                    nc.gpsimd.dma_start(
                        rbb[0:64, :], rr_d[0:1, :].partition_broadcast(64))
                win = slice(s * NT + tq * 512, s * NT + (tq + 1) * 512)
                if hh == 0:
                    nc.vector.scalar_tensor_tensor(
                        ob[0:64, p, win], pv[0:64, :], 0.0, rbb[0:64, :],
                        BYPASS, MULT)
                else:
                    stn = stgp.tile([64, 512], BF16, tag="stn", name="stn")
                    nc.vector.scalar_tensor_tensor(
                        stn[:], pv[0:64, :], 0.0, rbb[0:64, :],
                        BYPASS, MULT)
                    nc.gpsimd.dma_start(ob[64:128, p, win], stn[:])

            def proj_piece(s, tq, m0, m1):
                n = 2 * s + tq
                for m in range(m0, m1):
                    ps = mmps.tile([128, 512], F32, tag="mm")
                    for c in range(CH):
                        nc.tensor.matmul(
                            ps[:],
                            pw_sb[:, c, m * 128:(m + 1) * 128],
                            ob[:, c, n * 512:(n + 1) * 512],
                            start=(c == 0), stop=(c == CH - 1),
                        )
                    y_t = yp.tile([128, 512], F32)
                    nc.vector.tensor_scalar(
                        y_t[:], ps[:], pb_sb[:, m:m + 1], None, ADD)
                    nc.gpsimd.dma_start(
                        out[m * 128:(m + 1) * 128, n * 512:(n + 1) * 512],
                        y_t[:])

            # The tensor queue is in-order: when the head S-matmul waits for
            # an exp to free a PSUM slot, everything behind it stalls. So PV
            # and proj work is cut into ~2-4us pieces and fed between
            # S-matmul groups, where those stalls would otherwise idle the
            # tensor engine. Proj pieces for a tile are deferred by one
            # window (via projq) so the tile's normalize chains (several us
            # of DVE+DMA latency) have finished before the proj matmuls
            # reach the tensor-queue head.
            def item_pieces(item):
                if legacy_pv:
                    return [lambda: emit_pv_legacy(item)]
                return [lambda: emit_pv_v2_hh(item, 0),
                        lambda: emit_pv_v2_hh(item, 1)]

            def item_proj_pieces(item):
                if item[0] != NPAIR - 1 or phases != "all":
                    return []
                return [lambda: proj_piece(item[1], item[2], 0, 3),
                        lambda: proj_piece(item[1], item[2], 3, 6)]

            if phases == "v":
                for s in range(SPC):
                    for tt in range(8):
                        v_block(s, tt)
                return

            # V blocks become fillers; deadline = window of the first PV pop
            # that reads vbs[s] (pop of (p0, s, tq0) = append#(2s+1) + LAG)
            for s in range(SPC):
                for tt in range(8):
                    fillers.append((2 * s + 1 + LAG,
                                    lambda s=s, tt=tt: v_block(s, tt)))
            fillers.sort(key=lambda f: f[0])

            URGENT = ((0, 0), (1, 0), (1, 1))
            REST = ((0, 1), (1, 2), (1, 3), (0, 2), (0, 3))

            def pair_setup(p):
                wqk_t = wqkp.tile([128, CH, 256], BF16)
                nc.sync.dma_start(
                    wqk_t[:, :, 0:128],
                    wqk[:, p * 128:(p + 1) * 128].rearrange(
                        "(c p) n -> p c n", p=128))
                nc.sync.dma_start(
                    wqk_t[:, :, 128:256],
                    wqk[:, C + p * 128: C + (p + 1) * 128].rearrange(
                        "(c p) n -> p c n", p=128))
                qk_t = qkp.tile([128, 2, T], BF16)
                return wqk_t, qk_t

            pending = []
            work = []
            projq = []
            cur = pair_setup(0)
            for qk, n in URGENT:
                qk_group(0, cur[0], cur[1], qk, n)
            for p in range(NPAIR):
                wqk_t, qk_t = cur
                # non-urgent groups become fillers: q(n) needed at window
                # (p, n//2, n%2), k(n) needed at window (p, n//2, 0)
                for qk, n in REST:
                    ddl = wnum(p, n // 2, (n % 2) if qk == 0 else 0)
                    fillers.append((ddl,
                                    lambda p=p, w=wqk_t, q=qk_t, qk=qk, n=n:
                                    qk_group(p, w, q, qk, n)))
                # prefetch the NEXT pair's urgent groups as end-of-pair
                # fillers, so the pair boundary has no tensor burst that
                # would starve the scalar engine
                if p + 1 < NPAIR:
                    nxt = pair_setup(p + 1)
                    for i, (qk, n) in enumerate(URGENT):
                        fillers.append((4 * p + 2 + i,
                                        lambda w=nxt[0], q=nxt[1], qk=qk,
                                        n=n, pp=p + 1:
                                        qk_group(pp, w, q, qk, n)))
                else:
                    nxt = None
                fillers.sort(key=lambda f: f[0])

                if phases == "vqk":
                    cur = nxt
                    continue
                for s in range(SPC):
                    for tq in range(2):
                        w = wnum(p, s, tq)
                        drain(upto=w)  # correctness: qk deps of this window
                        if (phases not in ("vqkst", "nopv")
                                and len(pending) >= LAG):
                            item = pending.pop(0)
                            drain(upto=wnum(*item[:3]) + LAG)  # vbs deps
                            work.extend(projq)  # prior window's proj work
                            projq = item_proj_pieces(item)
                            work.extend(item_pieces(item))
                        pt = [ptp.tile([128, 8, 512], BF16, tag="pt",
                                       name=f"pt{_h}") for _h in range(2)]
                        for tk2 in range(4):
                            st2 = [stps.tile([128, 2, 512], F32, tag="st",
                                             name=f"st{_h}") for _h in range(2)]
                            for sub in range(2):
                                tk = 2 * tk2 + sub
                                for hh in range(2):
                                    lo = hh * 64
                                    nc.tensor.matmul(
                                        st2[hh][:, sub, :],
                                        qk_t[lo:lo + 64, 1,
                                             s * NT + tk * 128:
                                             s * NT + (tk + 1) * 128],
                                        qk_t[lo:lo + 64, 0,
                                             s * NT + tq * 512:
                                             s * NT + (tq + 1) * 512],
                                        start=True, stop=True,
                                        tile_position=(lo, 0),
                                    )
                            if phases != "vqkst":
                                for hh in range(2):
                                    nc.scalar.activation(
                                        pt[hh][:, 2 * tk2:2 * tk2 + 2, :],
                                        st2[hh][:], EXP, scale=SCALE)
                            if work:
                                work.pop(0)()
                            else:
                                drain(k=1)
                        if phases in ("vqkst", "nopv"):
                            continue
                        pending.append((p, s, tq, pt))
                cur = nxt
            while work:
                work.pop(0)()
            drain()
            while pending:
                item = pending.pop(0)
                for piece in item_pieces(item):
                    piece()
                projq.extend(item_proj_pieces(item))
            for piece in projq:
                piece()

        for _rep in range(reps):
            emit_rep()
    nc.compile()
    return nc


def make_in_maps(x, label, alpha, qkv_w, qkv_b, proj_w, proj_b):
    x = np.asarray(x, np.float32)
    label = np.asarray(label)
    alpha = np.asarray(alpha, np.float32)
    qkv_w = np.asarray(qkv_w, np.float32)
    qkv_b = np.asarray(qkv_b, np.float32)
    proj_w = np.asarray(proj_w, np.float32)
    proj_b = np.asarray(proj_b, np.float32)

    sig = 1.0 / (1.0 + np.exp(-alpha[label]))          # (B, 3C) f32
    wqkT = np.ascontiguousarray(qkv_w[:2 * C].T).astype(ml_dtypes.bfloat16)
    wvT = np.ascontiguousarray(qkv_w[2 * C:].T)         # (C, C) f32
    pw_bf = np.ascontiguousarray(proj_w.T).astype(ml_dtypes.bfloat16)
    pb_arr = np.ascontiguousarray(proj_b.reshape(CH, 128).T)

    in_maps = []
    for i in range(NCORES):
        sl = slice(SPC * i, SPC * (i + 1))
        xs = x[sl]                                      # (2, NT, C)
        xt = np.ascontiguousarray(
            xs.transpose(2, 0, 1).reshape(C, T)).astype(ml_dtypes.bfloat16)
        sig_i = sig[sl]                                 # (2, 3C)
        sqk = sig_i[:, :2 * C]                          # (2, 2C)
        sq = sqk.reshape(SPC, 12, 128).transpose(2, 0, 1).reshape(128, SPC * 12)
        bq = ((qkv_b[None, :2 * C] * sqk).reshape(SPC, 12, 128)
              .transpose(2, 0, 1).reshape(128, SPC * 12))
        sigbq_i = np.ascontiguousarray(np.concatenate([sq, bq], axis=1))
        sigv = sig_i[:, 2 * C:]                         # (2, C)
        wv_sc = wvT[None, :, :] * sigv[:, None, :]      # (2, C, C)
        wv_pad = np.zeros((SPC, C, CP), np.float32)
        bvs_pad = np.zeros((SPC, CP), np.float32)
        for h in range(H):
            wv_pad[:, :, h * 65:h * 65 + 64] = wv_sc[:, :, h * 64:(h + 1) * 64]
            bvs_pad[:, h * 65:h * 65 + 64] = (
                qkv_b[None, 2 * C + h * 64: 2 * C + (h + 1) * 64]
                * sigv[:, h * 64:(h + 1) * 64])
            bvs_pad[:, h * 65 + 64] = 1.0
        in_maps.append({
            "xt": xt, "wqk": wqkT,
            "wv": np.ascontiguousarray(wv_pad).astype(ml_dtypes.bfloat16),
            "sigbq": sigbq_i,
            "bvs": np.ascontiguousarray(bvs_pad).astype(ml_dtypes.bfloat16),
            "pw": pw_bf, "pb": pb_arr,
        })
    return in_maps


_NC = None
LAST_RESULT = None


def kernel(x, label, alpha, qkv_w, qkv_b, proj_w, proj_b):
    global _NC, LAST_RESULT
    if _NC is None:
        _NC = build()
    in_maps = make_in_maps(x, label, alpha, qkv_w, qkv_b, proj_w, proj_b)
    res = run_bass_kernel_spmd(_NC, in_maps, core_ids=list(range(NCORES)))
    LAST_RESULT = res
    outs = []
    for i in range(NCORES):
        y = np.asarray(res.results[i]["out"])           # (C, T)
        outs.append(y.reshape(C, SPC, NT).transpose(1, 2, 0))
    return np.ascontiguousarray(np.concatenate(outs, axis=0), dtype=np.float32)
